# revision 30
# baseline (speedup 1.0000x reference)
import sys

if "/opt/trn_rl_repo" not in sys.path:
    sys.path.insert(0, "/opt/trn_rl_repo")

from contextlib import ExitStack

import ml_dtypes
import numpy as np

import concourse.bacc as bacc
import concourse.bass as bass
import concourse.mybir as mybir
import concourse.tile as tile
from concourse.bass_utils import run_bass_kernel_spmd

B, H, N, T, D = 4, 4, 32, 96, 32
DQK = T * D  # 3072
SCALE = float(DQK**0.5)
NCORES = 8
NCH = DQK // 128  # 24 contraction chunks for the QK gram
NB = DQK // 512  # 6 psum column blocks
F32 = mybir.dt.float32
BF16 = mybir.dt.bfloat16
NEG = -1.0e30


def _build_program(NTF, R):
    """NTF full 128-row V tiles plus an optional leading R-row tile."""
    NTE = NTF + (1 if R else 0)  # logical tile count (R-tile is tile 0)
    nc = bacc.Bacc()
    qkt_d = nc.declare_dram_parameter("qkt", [128, NCH * 128], BF16, isOutput=False)
    mb_d = nc.declare_dram_parameter("mb", [32, 64], F32, isOutput=False)
    # V rows packed by (head, kept (i,j)); column block kt*3072 of v holds
    # full tile kt's 128 rows as partitions, so any tile span is one
    # contiguous DMA. The R-row remainder rides separately.
    v_d = nc.declare_dram_parameter("v", [128, NTF * DQK], BF16, isOutput=False)
    if R:
        vr_d = nc.declare_dram_parameter("vr", [R, DQK], BF16, isOutput=False)
    g_d = nc.declare_dram_parameter("g", [32, 2 * NTE * 128], BF16, isOutput=False)
    o_d = nc.declare_dram_parameter("o", [128, NTE * 64], BF16, isOutput=False)
    out_d = nc.declare_dram_parameter("out", [64, DQK], BF16, isOutput=True)

    # Full-tile DMA chunks (column ranges of v_d): one single tile first so
    # the matmul stream starts early, then pairs, with the final tile split
    # 4-banks/2-banks so most output copies overlap the last transfer.
    chunks = []
    if NTF == 1:
        chunks = [(0, 2048), (2048, DQK)]
    else:
        chunks.append((0, DQK))
        t0 = 1
        while t0 < NTF - 1:
            t1 = min(t0 + 2, NTF - 1)
            chunks.append((t0 * DQK, t1 * DQK))
            t0 = t1
        last = (NTF - 1) * DQK
        chunks.append((last, last + 2048))
        chunks.append((last + 2048, NTF * DQK))

    with tile.TileContext(nc) as tc, ExitStack() as ctx:
        sb = ctx.enter_context(tc.tile_pool(name="sb", bufs=1))
        vp = ctx.enter_context(tc.tile_pool(name="vp", bufs=1))
        pp = ctx.enter_context(tc.tile_pool(name="pp", bufs=1, space="PSUM"))

        qkt_sb = sb.tile([128, NCH * 128], BF16, tag="qkt")
        mb_sb = sb.tile([32, 64], F32, tag="mb")
        g_sb = sb.tile([32, 2 * NTE * 128], BF16, tag="g")
        o_sb = sb.tile([128, NTE * 64], BF16, tag="o")
        a2_sb = sb.tile([128, NTE * 64], BF16, tag="a2")
        t2_sb = sb.tile([32, 64], F32, tag="t2")
        et_sb = sb.tile([32, 64], BF16, tag="et")
        ones_sb = sb.tile([32, 1], BF16, tag="ones")
        ri_sb = sb.tile([64, 1], F32, tag="ri")
        out_sb = sb.tile([64, DQK], BF16, tag="out")

        # Everything phase A needs (qkt, g, o) plus the first V tile leads
        # the sync ring at full aggregate rate — the scalar ring is starved
        # to ~30 GB/s while the sync firehose runs, so anything
        # latency-critical must NOT sit there.
        half = NCH * 64
        nc.sync.dma_start(qkt_sb[:, 0:half], qkt_d[:, 0:half])
        nc.sync.dma_start(qkt_sb[:, half:], qkt_d[:, half:])
        nc.sync.dma_start(g_sb[:, :], g_d[:, :])
        nc.sync.dma_start(o_sb[:, :], o_d[:, :])
        if R:
            vr_sb = vp.tile([R, DQK], BF16, tag="vr")
            nc.sync.dma_start(vr_sb[:, :], vr_d[:, :])
        vts = []
        for lo, hi in chunks:
            vt = vp.tile([128, hi - lo], BF16, tag=f"v{lo}")
            nc.sync.dma_start(vt[:, :], v_d[:, lo:hi])
            vts.append(vt)

        nc.scalar.dma_start(mb_sb[:, :], mb_d[:, :])

        nc.vector.memset(ones_sb[:, :], 1.0)

        # HAM warmup: the PE clock-gate only opens after a full ~3.4us
        # activity window of sustained matmuls. Keep the PE busy on
        # throwaway matmuls (zeroed scratch, into a bank the real
        # accumulation later re-opens with start=True) from the preamble
        # until real work flows, so the stream matmuls all run at 2.4 GHz
        # instead of paying ~2.5us of cold 1.2 GHz starts.
        scr_sb = sb.tile([128, 512], BF16, tag="scr")
        nc.vector.memset(scr_sb[:, :], 0.0)

        def warmup(k):
            for _ in range(k):
                nc.tensor.matmul(
                    opst[NB - 1][:, :],
                    scr_sb[:, 0:64],
                    scr_sb[:, :],
                    start=True,
                    stop=True,
                )

        opst = [pp.tile([64, 512], F32, tag=f"p{n}", name=f"o{n}") for n in range(NB)]
        warmup(8)

        # Stacked transposed gram: [64,64] = Kstack^T @ Qstack, so diag
        # block t is score(i,j)^T for head t (j on partitions) and exp()
        # directly produces eT — no transposes.
        gram_t = pp.tile([64, 512], F32, tag="pa", name="gram")
        for c in range(NCH):
            sl = qkt_sb[:, 128 * c : 128 * (c + 1)]
            nc.tensor.matmul(
                gram_t[:, 0:64],
                sl[:, 64:128],
                sl[:, 0:64],
                start=(c == 0),
                stop=(c == NCH - 1),
            )

        # Scores ~N(0,1): exp never overflows f32, so no max-subtraction
        # (mask NEG underflows to exactly 0). Normalization is deferred to
        # the PSUM->SBUF output copies.
        for t_ in range(2):
            nc.vector.tensor_tensor(
                t2_sb[:, 32 * t_ : 32 * t_ + 32],
                gram_t[32 * t_ : 32 * t_ + 32, 32 * t_ : 32 * t_ + 32],
                mb_sb[:, 32 * t_ : 32 * t_ + 32],
                mybir.AluOpType.add,
            )
        nc.scalar.activation(
            et_sb[:, :],
            t2_sb[:, :],
            mybir.ActivationFunctionType.Exp,
            bias=0.0,
            scale=1.0 / SCALE,
        )

        warmup(3)

        # Softmax denominators: column sums of eT via ones-matmuls into the
        # bank freed by gram; the partition-shifted reciprocals stack the
        # per-head 1/rowsum onto output partitions 0:31 / 32:63.
        rs_t = pp.tile([32, 512], F32, tag="pa", name="rs")
        for t_ in range(2):
            nc.tensor.matmul(
                rs_t[:, t_ : t_ + 1],
                et_sb[:, 32 * t_ : 32 * t_ + 32],
                ones_sb[:, :],
                start=True,
                stop=True,
            )
        for t_ in range(2):
            nc.vector.reciprocal(
                ri_sb[32 * t_ : 32 * t_ + 32, :], rs_t[:, t_ : t_ + 1]
            )

        # Per-tile gather: X[p, 32t+i] = e_t[i, j_r(p)] via one-hot g;
        # a2 = X * o keeps the one (head, i_r(p)) == (t, i) entry per
        # packed V row. Emitted one chunk ahead of its consumers so the
        # vector mult hides behind the previous chunk's matmuls.
        built = set()

        def build_a2(kt):
            if kt in built:
                return
            built.add(kt)
            X_t = pp.tile([128, 512], F32, tag=("pb", "pa")[kt % 2], name=f"x{kt}")
            for t_ in range(2):
                gsl = g_sb[:, (NTE * t_ + kt) * 128 : (NTE * t_ + kt + 1) * 128]
                nc.tensor.matmul(
                    X_t[:, 32 * t_ : 32 * t_ + 32],
                    gsl,
                    et_sb[:, 32 * t_ : 32 * t_ + 32],
                    start=True,
                    stop=True,
                )
            nc.vector.tensor_tensor(
                a2_sb[:, 64 * kt : 64 * (kt + 1)],
                X_t[:, 0:64],
                o_sb[:, 64 * kt : 64 * (kt + 1)],
                mybir.AluOpType.mult,
            )

        # chunk tile indices in LOGICAL tile numbering (R-tile is 0)
        off = 1 if R else 0
        tiles_of = [
            list(range(lo // DQK + off, (hi + DQK - 1) // DQK + off))
            for lo, hi in chunks
        ]
        if R:
            build_a2(0)
        for kt in tiles_of[0]:
            build_a2(kt)
        warmup(2)
        # R-tile matmuls open every bank's accumulation group (start=True).
        if R:
            for n in range(NB):
                nc.tensor.matmul(
                    opst[n][:, :],
                    a2_sb[0:R, 0:64],
                    vr_sb[:, 512 * n : 512 * (n + 1)],
                    start=True,
                    stop=False,
                )
        for ci, (lo, hi) in enumerate(chunks):
            if ci + 1 < len(chunks):
                for kt in tiles_of[ci + 1]:
                    build_a2(kt)
            vt = vts[ci]
            for col in range(lo, hi, 512):
                kt = col // DQK + off
                n = (col % DQK) // 512
                nc.tensor.matmul(
                    opst[n][:, :],
                    a2_sb[:, 64 * kt : 64 * (kt + 1)],
                    vt[:, col - lo : col - lo + 512],
                    start=(kt == 0 and not R),
                    stop=(col // DQK == NTF - 1),
                )

        # Scale each finished bank by 1/rowsum into bf16 (copies alternate
        # engines and are never interrupted by DMA-issue work); the sync
        # ring, idle once V is in, streams the result out in three paired
        # pieces as copies complete.
        eng = [nc.scalar, nc.vector, nc.scalar, nc.vector, nc.scalar, nc.vector]
        for n in range(NB):
            dst = out_sb[:, 512 * n : 512 * (n + 1)]
            if eng[n] is nc.scalar:
                nc.scalar.mul(dst, opst[n][:, :], ri_sb[:, :])
            else:
                nc.vector.tensor_scalar_mul(dst, opst[n][:, :], ri_sb[:, :])
            if n == 2:
                nc.sync.dma_start(out_d[:, 0:1536], out_sb[:, 0:1536])
            elif n == 4:
                nc.sync.dma_start(out_d[:, 1536:2560], out_sb[:, 1536:2560])
            elif n == 5:
                nc.sync.dma_start(out_d[:, 2560:3072], out_sb[:, 2560:3072])

    nc.finalize()
    return nc


_PROGS = {}


def _get_program(NTF, R):
    if (NTF, R) not in _PROGS:
        _PROGS[(NTF, R)] = _build_program(NTF, R)
    return _PROGS[(NTF, R)]


def _assign_pairs(mask):
    """Balance kept-row totals: pair the largest with the smallest."""
    kept = np.asarray(mask).reshape(B * H, N * N).astype(np.int64).sum(axis=1)
    order = np.argsort(-kept, kind="stable")
    pairs = [(int(order[i]), int(order[2 * NCORES - 1 - i])) for i in range(NCORES)]
    rows = max(kept[a] + kept[b] for a, b in pairs)
    rows = max(32, -(-int(rows) // 32) * 32)  # pad to a multiple of 32
    NTF, R = divmod(rows, 128)
    if NTF == 0:  # tiny masks: use one full tile
        NTF, R = 1, 0
        rows = 128
    return pairs, NTF, R, rows


def make_in_maps(Q, K, V, mask, pairs, NTF, R):
    NTE = NTF + (1 if R else 0)
    Q = np.asarray(Q)
    K = np.asarray(K)
    V = np.asarray(V)
    mask = np.asarray(mask)
    in_maps = []
    for c in range(NCORES):
        bh = [(p // H, p % H) for p in pairs[c]]
        cols = [Q[b, h].T for b, h in bh] + [K[b, h].T for b, h in bh]
        stack = np.concatenate(cols, axis=1)  # [3072, 128]
        qkt = (
            np.ascontiguousarray(stack.reshape(NCH, 128, 128).transpose(1, 0, 2))
            .reshape(128, NCH * 128)
            .astype(ml_dtypes.bfloat16)
        )
        # mb[j, 32t+i]: transposed mask bias, head blocks in columns
        mb = np.concatenate(
            [
                np.where(mask[b, h].T == 0, np.float32(NEG), np.float32(0.0))
                for b, h in bh
            ],
            axis=1,
        ).astype(np.float32)
        vj = np.zeros((R + NTF * 128, DQK), ml_dtypes.bfloat16)
        g = np.zeros((32, 2 * NTE * 128), ml_dtypes.bfloat16)
        o = np.zeros((128, NTE * 64), ml_dtypes.bfloat16)
        r0 = 0
        for t_, (b, h) in enumerate(bh):
            v2full = np.ascontiguousarray(V[b, h].transpose(1, 0, 2, 3)).reshape(
                N * N, DQK
            )
            keep = np.nonzero(mask[b, h].reshape(-1) != 0)[0]
            kb = len(keep)
            vj[r0 : r0 + kb] = v2full[keep].astype(ml_dtypes.bfloat16)
            i_r = keep // N
            j_r = keep % N
            rr = np.arange(r0, r0 + kb)
            # logical tile index / within-tile row (R-tile first)
            if R:
                kt_ = np.where(rr < R, 0, (rr - R) // 128 + 1)
                p_ = np.where(rr < R, rr, (rr - R) % 128)
            else:
                kt_ = rr // 128
                p_ = rr % 128
            g[j_r, (NTE * t_ + kt_) * 128 + p_] = 1.0
            o[p_, 64 * kt_ + 32 * t_ + i_r] = 1.0
            r0 += kb
        im = {"qkt": qkt, "mb": mb, "g": g, "o": o}
        if R:
            im["vr"] = np.ascontiguousarray(vj[0:R])
        im["v"] = np.ascontiguousarray(
            vj[R:].reshape(NTF, 128, DQK).transpose(1, 0, 2)
        ).reshape(128, NTF * DQK)
        in_maps.append(im)
    return in_maps


def kernel(Q=None, K=None, V=None, mask=None, _trace=False, **_ignored):
    pairs, NTF, R, _rows = _assign_pairs(mask)
    in_maps = make_in_maps(Q, K, V, mask, pairs, NTF, R)
    nc = _get_program(NTF, R)
    res = run_bass_kernel_spmd(nc, in_maps, list(range(NCORES)), trace=_trace)
    out = np.zeros((B, H, N, T, D), np.float32)
    for c in range(NCORES):
        r = np.asarray(res.results[c]["out"]).astype(np.float32)  # [64, 3072]
        for t_, p in enumerate(pairs[c]):
            b, h = p // H, p % H
            out[b, h] = r[32 * t_ : 32 * t_ + 32].reshape(N, T, D)
    if _trace:
        return out, res
    return out


# revision 31
# speedup vs baseline: 1.1004x; 1.1004x over previous
import sys

if "/opt/trn_rl_repo" not in sys.path:
    sys.path.insert(0, "/opt/trn_rl_repo")

from contextlib import ExitStack

import ml_dtypes
import numpy as np

import concourse.bacc as bacc
import concourse.bass as bass
import concourse.mybir as mybir
import concourse.tile as tile
from concourse.bass_utils import run_bass_kernel_spmd

B, H, N, T, D = 4, 4, 32, 96, 32
DQK = T * D  # 3072
SCALE = float(DQK**0.5)
NCORES = 8
NCH = DQK // 128  # 24 contraction chunks for the QK gram
NB = DQK // 512  # 6 psum column blocks
F32 = mybir.dt.float32
BF16 = mybir.dt.bfloat16
NEG = -1.0e30


def _build_program(NTF, R):
    """NTF full 128-row V tiles plus an optional leading R-row tile."""
    NTE = NTF + (1 if R else 0)  # logical tile count (R-tile is tile 0)
    nc = bacc.Bacc()
    qkt_d = nc.declare_dram_parameter("qkt", [128, NCH * 128], BF16, isOutput=False)
    mb_d = nc.declare_dram_parameter("mb", [32, 64], F32, isOutput=False)
    # V rows packed by (head, kept (i,j)); column block kt*3072 of v holds
    # full tile kt's 128 rows as partitions, so any tile span is one
    # contiguous DMA. The R-row remainder rides separately.
    v_d = nc.declare_dram_parameter("v", [128, NTF * DQK], BF16, isOutput=False)
    if R:
        vr_d = nc.declare_dram_parameter("vr", [R, DQK], BF16, isOutput=False)
    g_d = nc.declare_dram_parameter("g", [32, 2 * NTE * 128], BF16, isOutput=False)
    o_d = nc.declare_dram_parameter("o", [128, NTE * 64], BF16, isOutput=False)
    out_d = nc.declare_dram_parameter("out", [64, DQK], BF16, isOutput=True)

    # Full-tile DMA chunks (column ranges of v_d): one single tile first so
    # the matmul stream starts early, then pairs, with the final tile split
    # 4-banks/2-banks so most output copies overlap the last transfer.
    chunks = []
    if NTF == 1:
        chunks = [(0, 2048), (2048, DQK)]
    else:
        chunks.append((0, DQK))
        t0 = 1
        while t0 < NTF - 1:
            t1 = min(t0 + 2, NTF - 1)
            chunks.append((t0 * DQK, t1 * DQK))
            t0 = t1
        last = (NTF - 1) * DQK
        chunks.append((last, last + 2048))
        chunks.append((last + 2048, NTF * DQK))

    with tile.TileContext(nc) as tc, ExitStack() as ctx:
        sb = ctx.enter_context(tc.tile_pool(name="sb", bufs=1))
        vp = ctx.enter_context(tc.tile_pool(name="vp", bufs=1))
        pp = ctx.enter_context(tc.tile_pool(name="pp", bufs=1, space="PSUM"))

        qkt_sb = sb.tile([128, NCH * 128], BF16, tag="qkt")
        mb_sb = sb.tile([32, 64], F32, tag="mb")
        g_sb = sb.tile([32, 2 * NTE * 128], BF16, tag="g")
        o_sb = sb.tile([128, NTE * 64], BF16, tag="o")
        a2_sb = sb.tile([128, NTE * 64], BF16, tag="a2")
        t2_sb = sb.tile([32, 64], F32, tag="t2")
        et_sb = sb.tile([32, 64], BF16, tag="et")
        ones_sb = sb.tile([32, 1], BF16, tag="ones")
        ri_sb = sb.tile([64, 1], F32, tag="ri")
        out_sb = sb.tile([64, DQK], BF16, tag="out")

        # Everything phase A needs (qkt, g, o) plus the first V tile leads
        # the sync ring at full aggregate rate — the scalar ring is starved
        # to ~30 GB/s while the sync firehose runs, so anything
        # latency-critical must NOT sit there.
        half = NCH * 64
        nc.sync.dma_start(qkt_sb[:, 0:half], qkt_d[:, 0:half])
        nc.sync.dma_start(qkt_sb[:, half:], qkt_d[:, half:])
        nc.sync.dma_start(g_sb[:, :], g_d[:, :])
        nc.sync.dma_start(o_sb[:, :], o_d[:, :])
        if R:
            vr_sb = vp.tile([R, DQK], BF16, tag="vr")
            nc.sync.dma_start(vr_sb[:, :], vr_d[:, :])
        vts = []
        for lo, hi in chunks:
            vt = vp.tile([128, hi - lo], BF16, tag=f"v{lo}")
            nc.sync.dma_start(vt[:, :], v_d[:, lo:hi])
            vts.append(vt)

        nc.scalar.dma_start(mb_sb[:, :], mb_d[:, :])

        nc.vector.memset(ones_sb[:, :], 1.0)

        # Stacked transposed gram: [64,64] = Kstack^T @ Qstack, so diag
        # block t is score(i,j)^T for head t (j on partitions) and exp()
        # directly produces eT — no transposes.
        gram_t = pp.tile([64, 512], F32, tag="pa", name="gram")
        for c in range(NCH):
            sl = qkt_sb[:, 128 * c : 128 * (c + 1)]
            nc.tensor.matmul(
                gram_t[:, 0:64],
                sl[:, 64:128],
                sl[:, 0:64],
                start=(c == 0),
                stop=(c == NCH - 1),
            )

        # Scores ~N(0,1): exp never overflows f32, so no max-subtraction
        # (mask NEG underflows to exactly 0). Normalization is deferred to
        # the PSUM->SBUF output copies.
        for t_ in range(2):
            nc.vector.tensor_tensor(
                t2_sb[:, 32 * t_ : 32 * t_ + 32],
                gram_t[32 * t_ : 32 * t_ + 32, 32 * t_ : 32 * t_ + 32],
                mb_sb[:, 32 * t_ : 32 * t_ + 32],
                mybir.AluOpType.add,
            )
        nc.scalar.activation(
            et_sb[:, :],
            t2_sb[:, :],
            mybir.ActivationFunctionType.Exp,
            bias=0.0,
            scale=1.0 / SCALE,
        )

        # Softmax denominators: column sums of eT via ones-matmuls into the
        # bank freed by gram; the partition-shifted reciprocals stack the
        # per-head 1/rowsum onto output partitions 0:31 / 32:63.
        rs_t = pp.tile([32, 512], F32, tag="pa", name="rs")
        for t_ in range(2):
            nc.tensor.matmul(
                rs_t[:, t_ : t_ + 1],
                et_sb[:, 32 * t_ : 32 * t_ + 32],
                ones_sb[:, :],
                start=True,
                stop=True,
            )
        for t_ in range(2):
            nc.vector.reciprocal(
                ri_sb[32 * t_ : 32 * t_ + 32, :], rs_t[:, t_ : t_ + 1]
            )

        # Per-tile gather: X[p, 32t+i] = e_t[i, j_r(p)] via one-hot g;
        # a2 = X * o keeps the one (head, i_r(p)) == (t, i) entry per
        # packed V row. Emitted one chunk ahead of its consumers so the
        # vector mult hides behind the previous chunk's matmuls.
        built = set()

        def build_a2(kt):
            if kt in built:
                return
            built.add(kt)
            X_t = pp.tile([128, 512], F32, tag=("pb", "pa")[kt % 2], name=f"x{kt}")
            for t_ in range(2):
                gsl = g_sb[:, (NTE * t_ + kt) * 128 : (NTE * t_ + kt + 1) * 128]
                nc.tensor.matmul(
                    X_t[:, 32 * t_ : 32 * t_ + 32],
                    gsl,
                    et_sb[:, 32 * t_ : 32 * t_ + 32],
                    start=True,
                    stop=True,
                )
            nc.vector.tensor_tensor(
                a2_sb[:, 64 * kt : 64 * (kt + 1)],
                X_t[:, 0:64],
                o_sb[:, 64 * kt : 64 * (kt + 1)],
                mybir.AluOpType.mult,
            )

        # Output accumulators: odd banks sit on PSUM partitions 64:128 so
        # each adjacent (even, odd) matmul pair lands in distinct PE column
        # groups and runs concurrently — the stream matmul time halves and
        # the PE outpaces DMA even when HAM-throttled.
        op_t = [pp.tile([128, 512], F32, tag=f"p{n}", name=f"o{n}") for n in range(NB)]
        opst = [
            op_t[n][0:64, :] if n % 2 == 0 else op_t[n][64:128, :] for n in range(NB)
        ]

        # chunk tile indices in LOGICAL tile numbering (R-tile is 0)
        off = 1 if R else 0
        tiles_of = [
            list(range(lo // DQK + off, (hi + DQK - 1) // DQK + off))
            for lo, hi in chunks
        ]
        if R:
            build_a2(0)
        for kt in tiles_of[0]:
            build_a2(kt)
        # R-tile matmuls open every bank's accumulation group (start=True).
        if R:
            for n in range(NB):
                nc.tensor.matmul(
                    opst[n][:, :],
                    a2_sb[0:R, 0:64],
                    vr_sb[:, 512 * n : 512 * (n + 1)],
                    start=True,
                    stop=False,
                )
        for ci, (lo, hi) in enumerate(chunks):
            if ci + 1 < len(chunks):
                for kt in tiles_of[ci + 1]:
                    build_a2(kt)
            vt = vts[ci]
            for col in range(lo, hi, 512):
                kt = col // DQK + off
                n = (col % DQK) // 512
                nc.tensor.matmul(
                    opst[n][:, :],
                    a2_sb[:, 64 * kt : 64 * (kt + 1)],
                    vt[:, col - lo : col - lo + 512],
                    start=(kt == 0 and not R),
                    stop=(col // DQK == NTF - 1),
                )

        # Scale each finished bank by 1/rowsum into bf16 (copies alternate
        # engines and are never interrupted by DMA-issue work); the sync
        # ring, idle once V is in, streams the result out in three paired
        # pieces as copies complete.
        eng = [nc.scalar, nc.vector, nc.scalar, nc.vector, nc.scalar, nc.vector]
        for n in range(NB):
            dst = out_sb[:, 512 * n : 512 * (n + 1)]
            if eng[n] is nc.scalar:
                nc.scalar.mul(dst, opst[n][:, :], ri_sb[:, :])
            else:
                nc.vector.tensor_scalar_mul(dst, opst[n][:, :], ri_sb[:, :])
            if n == 2:
                nc.sync.dma_start(out_d[:, 0:1536], out_sb[:, 0:1536])
            elif n == 4:
                nc.sync.dma_start(out_d[:, 1536:2560], out_sb[:, 1536:2560])
            elif n == 5:
                nc.sync.dma_start(out_d[:, 2560:3072], out_sb[:, 2560:3072])

    nc.finalize()
    return nc


_PROGS = {}


def _get_program(NTF, R):
    if (NTF, R) not in _PROGS:
        _PROGS[(NTF, R)] = _build_program(NTF, R)
    return _PROGS[(NTF, R)]


def _assign_pairs(mask):
    """Balance kept-row totals: pair the largest with the smallest."""
    kept = np.asarray(mask).reshape(B * H, N * N).astype(np.int64).sum(axis=1)
    order = np.argsort(-kept, kind="stable")
    pairs = [(int(order[i]), int(order[2 * NCORES - 1 - i])) for i in range(NCORES)]
    rows = max(kept[a] + kept[b] for a, b in pairs)
    rows = max(32, -(-int(rows) // 32) * 32)  # pad to a multiple of 32
    NTF, R = divmod(rows, 128)
    if NTF == 0:  # tiny masks: use one full tile
        NTF, R = 1, 0
        rows = 128
    return pairs, NTF, R, rows


def make_in_maps(Q, K, V, mask, pairs, NTF, R):
    NTE = NTF + (1 if R else 0)
    Q = np.asarray(Q)
    K = np.asarray(K)
    V = np.asarray(V)
    mask = np.asarray(mask)
    in_maps = []
    for c in range(NCORES):
        bh = [(p // H, p % H) for p in pairs[c]]
        cols = [Q[b, h].T for b, h in bh] + [K[b, h].T for b, h in bh]
        stack = np.concatenate(cols, axis=1)  # [3072, 128]
        qkt = (
            np.ascontiguousarray(stack.reshape(NCH, 128, 128).transpose(1, 0, 2))
            .reshape(128, NCH * 128)
            .astype(ml_dtypes.bfloat16)
        )
        # mb[j, 32t+i]: transposed mask bias, head blocks in columns
        mb = np.concatenate(
            [
                np.where(mask[b, h].T == 0, np.float32(NEG), np.float32(0.0))
                for b, h in bh
            ],
            axis=1,
        ).astype(np.float32)
        vj = np.zeros((R + NTF * 128, DQK), ml_dtypes.bfloat16)
        g = np.zeros((32, 2 * NTE * 128), ml_dtypes.bfloat16)
        o = np.zeros((128, NTE * 64), ml_dtypes.bfloat16)
        r0 = 0
        for t_, (b, h) in enumerate(bh):
            v2full = np.ascontiguousarray(V[b, h].transpose(1, 0, 2, 3)).reshape(
                N * N, DQK
            )
            keep = np.nonzero(mask[b, h].reshape(-1) != 0)[0]
            kb = len(keep)
            vj[r0 : r0 + kb] = v2full[keep].astype(ml_dtypes.bfloat16)
            i_r = keep // N
            j_r = keep % N
            rr = np.arange(r0, r0 + kb)
            # logical tile index / within-tile row (R-tile first)
            if R:
                kt_ = np.where(rr < R, 0, (rr - R) // 128 + 1)
                p_ = np.where(rr < R, rr, (rr - R) % 128)
            else:
                kt_ = rr // 128
                p_ = rr % 128
            g[j_r, (NTE * t_ + kt_) * 128 + p_] = 1.0
            o[p_, 64 * kt_ + 32 * t_ + i_r] = 1.0
            r0 += kb
        im = {"qkt": qkt, "mb": mb, "g": g, "o": o}
        if R:
            im["vr"] = np.ascontiguousarray(vj[0:R])
        im["v"] = np.ascontiguousarray(
            vj[R:].reshape(NTF, 128, DQK).transpose(1, 0, 2)
        ).reshape(128, NTF * DQK)
        in_maps.append(im)
    return in_maps


def kernel(Q=None, K=None, V=None, mask=None, _trace=False, **_ignored):
    pairs, NTF, R, _rows = _assign_pairs(mask)
    in_maps = make_in_maps(Q, K, V, mask, pairs, NTF, R)
    nc = _get_program(NTF, R)
    res = run_bass_kernel_spmd(nc, in_maps, list(range(NCORES)), trace=_trace)
    out = np.zeros((B, H, N, T, D), np.float32)
    for c in range(NCORES):
        r = np.asarray(res.results[c]["out"]).astype(np.float32)  # [64, 3072]
        for t_, p in enumerate(pairs[c]):
            b, h = p // H, p % H
            out[b, h] = r[32 * t_ : 32 * t_ + 32].reshape(N, T, D)
    if _trace:
        return out, res
    return out
